# revision 5
# baseline (speedup 1.0000x reference)
"""Q4_0-quantized linear y = x @ dequant(W).T on 8 Trainium2 cores.

2D sharding: 2-way over out_features (halves of 5504) x 4-way over the
contraction dim (quarters of 1024).  Core c -> (o_half = c//4, k_quarter
= c%4).  Each core computes a partial GEMM x[:, kq] @ W[oh, kq].T; the
host sums the 4 k-partials per o-half and concatenates the halves.

Host-side prep does the full Q4_0 dequantization (bit-exact with the
reference: int4 nibbles -> fp16 * fp16 group scale, rounded in fp16) and
pure layout repacking; the device runs a plain fp16 GEMM:

  - weights STATIONARY: lhsT tile [128 k, 128 o] per (k-tile, o-block);
    43 o-blocks x 8 k-tiles per core, streamed from DRAM per o-block
    (no on-device dequant phase to serialize behind).
  - x MOVING, fully SBUF-resident per core (16 MB): 16 tiles [128, 4096].
  - psum [128 o, 512 m] x 8 banks accumulated over the 8 k-tiles; the 8
    m-chunk matmuls per (o-block, k) share one stationary load.
  - redundant InstLdweights are deduped (tile_legalize emits one per
    matmul and walrus runs with --enable-ldw-opt=false, so each of the
    5504 matmuls would otherwise reload the PE array): ~700 loads remain,
    one per (m-group, o-block, k).
  - psum->SBUF drains alternate between the DVE and Activation engines
    (GPSIMD cannot access PSUM) so bank reuse never stalls the PE.

Loop order: m-group OUTER (the first o-block sweep only needs the first
8 MB of x -> ~no startup stall), then o-block, k, m-chunk.
"""

import numpy as np

import concourse.bass as bass
import concourse.bacc as bacc
import concourse.mybir as mybir
from concourse import tile
from concourse.bass_utils import run_bass_kernel_spmd

GROUP = 64
OUT_F, IN_F = 11008, 4096
B, S = 4, 2048
M = B * S                      # 8192 rows of x
NCORES = 8
O_WAYS, K_WAYS = 2, 4
O_SHARD = OUT_F // O_WAYS      # 5504 output features per core
K_SHARD = IN_F // K_WAYS       # 1024 contraction features per core
OB = O_SHARD // 128            # 43 o-blocks
KT = K_SHARD // 128            # 8 k-tiles
MG = 2                         # m-groups of 4096 (8 psum chunks of 512)
MGW = M // MG                  # 4096


def dedup_ldweights(nc):
    """Drop redundant InstLdweights: tile_legalize emits one per matmul, so
    consecutive matmuls sharing the same stationary operand reload the PE
    array needlessly.  A reload is redundant iff its weights AP matches the
    immediately preceding InstLdweights in the same block and it carries no
    semaphore waits/updates (a content-refresh of a reused tile buffer
    always carries the refresh-DMA wait, so it is preserved).  Ldweights
    never increment engine semaphores, so removal cannot shift downstream
    sem wait targets."""
    removed = 0
    for blk in nc.main_func.blocks:
        sig = None
        keep = []
        for inst in blk.instructions:
            if isinstance(inst, mybir.InstLdweights):
                ap = inst.ins[0]
                s = (
                    getattr(ap, "memref", None),
                    getattr(ap, "offset", None),
                    repr(getattr(ap, "ap", None)),
                    getattr(ap, "dtype", None),
                    inst.perf_mode,
                    inst.is_transpose,
                )
                si = inst.sync_info
                clean = si is None or (
                    len(si.on_wait) == 0 and len(si.on_update) == 0
                )
                if sig is not None and s == sig and clean:
                    removed += 1
                    continue
                sig = s
            elif isinstance(inst, mybir.InstMatmult):
                pass  # uses but does not modify the PE weight registers
            elif getattr(inst, "engine", None) == mybir.EngineType.PE:
                sig = None  # any other PE instruction: be conservative
            keep.append(inst)
        blk.instructions[:] = keep
    return removed


def build_program(repeat=1):
    """Single-core Bass program (SPMD: same program on all cores).

    repeat>1 wraps the kernel in an on-device loop for wall-clock delta
    timing (dispatch/transfer costs cancel between repeat counts)."""
    nc = bacc.Bacc(
        "TRN2", target_bir_lowering=False, debug=False, num_devices=NCORES
    )
    dt = mybir.dt

    # xr[mg*KT + t, p, j] = x[mg*4096 + j, kq0 + t*128 + p]
    xr = nc.dram_tensor(
        "xr", [MG * KT, 128, MGW], dt.float16, kind="ExternalInput"
    )
    # wt[ob, p, k*128 + o] = deq[oh0 + ob*128 + o, kq0 + k*128 + p]
    wt = nc.dram_tensor("wt", [OB, 128, KT * 128], dt.float16, kind="ExternalInput")
    # y[o, m] partial (fp16) for this core's (o_half, k_quarter)
    y = nc.dram_tensor("y", [O_SHARD, M], dt.float16, kind="ExternalOutput")

    with tile.TileContext(nc) as tc:
        with (
            tc.tile_pool(name="xp", bufs=1) as xp,
            tc.tile_pool(name="wp", bufs=3) as wp,
            tc.tile_pool(name="op", bufs=2) as op,
            tc.tile_pool(name="ps", bufs=1, space="PSUM") as ps,
        ):

            def body():
                # prefetch the first weight tile ahead of the bulk x DMAs so
                # the first ldweights isn't queued behind 16 MB of x traffic
                w_first = wp.tile([128, KT * 128], dt.float16, name="w", tag="w")
                nc.sync.dma_start(w_first[:], wt[0])
                xts = []
                for i in range(MG * KT):
                    xt = xp.tile([128, MGW], dt.float16, name=f"x{i}", tag=f"x{i}")
                    nc.sync.dma_start(xt[:], xr[i])
                    xts.append(xt)
                for mg in range(MG):
                    for ob in range(OB):
                        if mg == 0 and ob == 0:
                            wsb = w_first
                        else:
                            wsb = wp.tile(
                                [128, KT * 128], dt.float16, name="w", tag="w"
                            )
                            nc.sync.dma_start(wsb[:], wt[ob])
                        pss = [
                            ps.tile(
                                [128, 512], dt.float32, name=f"p{c}", tag=f"p{c}"
                            )
                            for c in range(8)
                        ]
                        for k in range(KT):
                            lhs = wsb[:, k * 128 : (k + 1) * 128]
                            xt = xts[mg * KT + k]
                            for c in range(8):
                                nc.tensor.matmul(
                                    pss[c][:],
                                    lhs,
                                    xt[:, c * 512 : (c + 1) * 512],
                                    start=(k == 0),
                                    stop=(k == KT - 1),
                                )
                        outt = op.tile([128, MGW], dt.float16, tag="o")
                        for c in range(8):
                            # alternate engines so psum banks drain in chunk
                            # order ~2x faster than one engine could
                            if c % 2 == 0:
                                nc.vector.tensor_copy(
                                    outt[:, c * 512 : (c + 1) * 512], pss[c][:]
                                )
                            else:
                                nc.scalar.copy(
                                    outt[:, c * 512 : (c + 1) * 512], pss[c][:]
                                )
                        nc.sync.dma_start(
                            y[
                                ob * 128 : (ob + 1) * 128,
                                mg * MGW : (mg + 1) * MGW,
                            ],
                            outt[:],
                        )

            if repeat > 1:
                with tc.For_i(0, repeat, 1):
                    body()
            else:
                body()

    dedup_ldweights(nc)
    nc.compile()
    return nc


def _dequant_fp16(linear_w, linear_s):
    """Bit-exact replica of reference.dequantize_q40 in numpy fp16."""
    w = np.asarray(linear_w, dtype=np.int8)       # [OUT_F*32, 64] packed
    s = np.asarray(linear_s, dtype=np.float16)    # [OUT_F*64, 1]
    msb = w >> 4
    lsb = (w << 4) >> 4
    full = np.concatenate([msb, lsb], axis=1).reshape(-1, GROUP)  # [N, 64]
    return (full.astype(np.float16) * s).reshape(OUT_F, IN_F)     # fp16


def prep_inputs(x, linear_w, linear_s):
    """Host-side dequantization + layout repacking -> per-core input maps."""
    deq = _dequant_fp16(linear_w, linear_s)           # [OUT_F, IN_F] fp16
    x2 = np.asarray(x, dtype=np.float16).reshape(M, IN_F)

    in_maps = []
    for c in range(NCORES):
        h, q = c // K_WAYS, c % K_WAYS
        dq_c = deq[h * O_SHARD : (h + 1) * O_SHARD,
                   q * K_SHARD : (q + 1) * K_SHARD]    # [5504, 1024]
        wtc = np.ascontiguousarray(
            dq_c.reshape(OB, 128, KT, 128).transpose(0, 3, 2, 1)
        ).reshape(OB, 128, KT * 128)
        xc = x2[:, q * K_SHARD : (q + 1) * K_SHARD]    # [8192, 1024]
        xrc = np.ascontiguousarray(
            xc.reshape(MG, MGW, KT, 128).transpose(0, 2, 3, 1)
        ).reshape(MG * KT, 128, MGW)
        in_maps.append({"xr": xrc, "wt": wtc})
    return in_maps


_CACHED = {}


def kernel(x, linear_w, linear_s):
    if "nc" not in _CACHED:
        _CACHED["nc"] = build_program()
    nc = _CACHED["nc"]
    in_maps = prep_inputs(x, linear_w, linear_s)
    res = run_bass_kernel_spmd(nc, in_maps, list(range(NCORES)))
    halves = []
    for h in range(O_WAYS):
        acc = np.zeros((O_SHARD, M), dtype=np.float32)
        for q in range(K_WAYS):
            acc += res.results[h * K_WAYS + q]["y"].astype(np.float32)
        halves.append(acc)
    y_full = np.concatenate(halves, axis=0)            # [OUT_F, M] fp32
    return np.ascontiguousarray(y_full.T).astype(np.float16).reshape(B, S, OUT_F)
